# revision 17
# baseline (speedup 1.0000x reference)
"""Trainium2 Bass kernel for DecoderAttention (B=16, T=1024, D=1024, H=16).

Sharding: pure data-parallel over batch — 16 batch items / 8 cores = 2 per
core. No collectives.

Design (HW-microbenchmarked, see microbench.py):
  * All matmul operands are bf16 (weights converted once at startup and kept
    resident in SBUF; hs converted on load by the ACT engine). PSUM stays f32.
  * AV uses the "natural" formulation: out[q, d] = attn^T @ V with the exp
    output as the *stationary* operand ([128,128] bf16 slices -> fast weight
    load at 2 elem/cycle/partition) and V~ ([128, 64+1] with a fused ones
    column producing the softmax denominator) as the moving operand. Measured
    1.54x faster than streaming the attention matrix through the moving port.
  * Softmax normalization is a per-partition reciprocal + tensor_scalar
    multiply on DVE (the natural layout puts q on partitions) — the PE
    broadcast matmuls of the old scheme are gone.
  * attn_out [q, d] pairs (two heads side by side = 128 cols) are PE-
    transposed back to attnT [d, q] for the out-projection stationary.
  * QK^T logits keep the two-heads-row-group-paired K=64 matmuls (measured:
    two such matmuls share one 512-col stream window).
"""

import os
import sys

import numpy as np

sys.path.insert(0, "/opt/trn_rl_repo")

import concourse.bass as bass  # noqa: E402
import concourse.mybir as mybir  # noqa: E402
import concourse.tile as tile  # noqa: E402
from concourse import bacc  # noqa: E402
from concourse.bass_utils import run_bass_kernel_spmd  # noqa: E402
from concourse.masks import make_identity  # noqa: E402

F32 = mybir.dt.float32
F32R = mybir.dt.float32r
BF16 = mybir.dt.bfloat16

B, T, D = 16, 1024, 1024
H, HD = 16, 64
N_CORES = 8
BL = B // N_CORES  # batch items per core
P = 128
CT = D // P  # contraction tiles (8)
TT = T // P  # token tiles (8)
NQ = 512  # matmul moving free dim for big streams
HDe = HD + 1  # head dim + denominator column
SCALE = 1.0 / np.sqrt(HD)

_last_results = None  # test.py reads this for the profile

Exp = mybir.ActivationFunctionType.Exp
Copy = mybir.ActivationFunctionType.Copy
ADD = mybir.AluOpType.add
MULT = mybir.AluOpType.mult


def build_program(time_loops=None, upto=None):
    # time_loops=R builds a timing variant: the whole per-item pipeline runs
    # R times inside a hardware loop (outputs overwritten each pass), letting
    # the amortized-dispatch timer divide away RPC noise.
    # upto: phase ladder for HW attribution (requires time_loops):
    #   'hsT' | 'vproj' | 'qk' | 'logits' | 'av' | None (full)
    nc = bacc.Bacc(
        "TRN2", target_bir_lowering=False, debug=False, num_devices=N_CORES
    )

    hs = nc.dram_tensor("hidden_states", [BL, T, D], F32, kind="ExternalInput")
    w_qkv = nc.dram_tensor("w_qkv", [D, 3 * D], F32, kind="ExternalInput")
    b_qkv = nc.dram_tensor("b_qkv", [3 * D], F32, kind="ExternalInput")
    w_out = nc.dram_tensor("w_out", [D, D], F32, kind="ExternalInput")
    b_out = nc.dram_tensor("b_out", [D], F32, kind="ExternalInput")
    out = nc.dram_tensor("out", [BL, T, D], F32, kind="ExternalOutput")

    with tile.TileContext(nc) as tc:
        with (
            tc.tile_pool(name="consts", bufs=1) as consts,
            tc.tile_pool(name="wts", bufs=1) as wts,
            tc.tile_pool(name="main", bufs=1) as main,
            tc.tile_pool(name="pipe", bufs=2) as pipe,
            tc.tile_pool(name="psum", bufs=1, space="PSUM") as psum,
        ):
            # ---------------- constants ----------------
            id_stg = pipe.tile([P, NQ], F32, tag="idstg", bufs=1,
                               name="id_stg")
            make_identity(nc, id_stg[:, 0:P])
            ident_bf = consts.tile([P, P], BF16)
            nc.vector.tensor_copy(ident_bf, id_stg[:, 0:P])
            ones_ph = consts.tile([P, H, 1], BF16)
            nc.gpsimd.memset(ones_ph, 1.0)
            ones_row = consts.tile([1, P], F32)
            nc.gpsimd.memset(ones_row, 1.0)
            # per-partition bias for QT/KT tiles: bq[p, jt] = b_qkv[jt*128+p]
            bq = consts.tile([P, 2 * CT], F32)
            nc.sync.dma_start(
                out=bq, in_=b_qkv.rearrange("(i p) -> p i", p=P)[:, 0 : 2 * CT]
            )
            # broadcast b_qkv V-slice and b_out along partitions via K=1 matmul
            bcast_bv = consts.tile([P, D], BF16)
            bcast_bout = consts.tile([P, D], BF16)

            def emit_bias_bcasts():
                bv_row = pipe.tile([1, D], F32, tag="ot", bufs=2, name="bv_row")
                nc.sync.dma_start(out=bv_row, in_=b_qkv[2 * D : 3 * D][None, :])
                bout_row = pipe.tile([1, D], F32, tag="ot", bufs=2,
                                     name="bout_row")
                nc.sync.dma_start(out=bout_row, in_=b_out[None, :])
                for dst, src in ((bcast_bv, bv_row), (bcast_bout, bout_row)):
                    ps_b = psum.tile([P, D], F32, tag="p_l", bufs=2)
                    for cc in range(2):
                        sl = slice(cc * NQ, (cc + 1) * NQ)
                        nc.tensor.matmul(
                            ps_b[:, sl], ones_row, src[:, sl],
                            start=True, stop=True,
                        )
                    nc.vector.tensor_copy(dst, ps_b)

            emit_bias_bcasts()

            # ------- weights -> bf16 resident (converted via hs staging) ---
            # wqk_bf[c][p, jt, j]: stationary slices for the QK projection
            # (jt 0..7 = Q j-tiles, 8..15 = K j-tiles)
            wqk_bf = [
                wts.tile([P, 2 * CT, P], BF16, name=f"wqk{c}") for c in range(CT)
            ]
            wv_bf = [wts.tile([P, D], BF16, name=f"wv{c}") for c in range(CT)]
            wout_bf = [wts.tile([P, D], BF16, name=f"wo{c}") for c in range(CT)]
            wq_resh = w_qkv.rearrange("(c p) j -> p c j", p=P)

            def stage_convert(dst_ap, src_ap, i, name):
                # route a [P, NQ] f32 chunk through the hs staging tags
                stg = pipe.tile([P, NQ], F32, tag=f"hsl{i % 4}", bufs=2,
                                name=f"wst_{name}")
                (nc.sync if i % 2 == 0 else nc.gpsimd).dma_start(
                    out=stg, in_=src_ap
                )
                nc.scalar.activation(dst_ap, stg, Copy)

            def emit_wv_conv():
                for c in range(CT):
                    for q in range(2):
                        sl = slice(q * NQ, (q + 1) * NQ)
                        stage_convert(
                            wv_bf[c][:, sl],
                            w_qkv[c * P : (c + 1) * P, 2 * D + q * NQ :
                                  2 * D + (q + 1) * NQ],
                            2 * c + q, f"wv{c}_{q}",
                        )

            def emit_wqk_conv():
                for c in range(CT):
                    w3 = wqk_bf[c].rearrange("p jt j -> p (jt j)")
                    for q in range(4):
                        sl = slice(q * NQ, (q + 1) * NQ)
                        stage_convert(
                            w3[:, sl], wq_resh[:, c, q * NQ : (q + 1) * NQ],
                            4 * c + q, f"wqk{c}_{q}",
                        )

            def emit_wout_conv(cs):
                for c in cs:
                    for q in range(2):
                        sl = slice(q * NQ, (q + 1) * NQ)
                        stage_convert(
                            wout_bf[c][:, sl],
                            w_out[c * P : (c + 1) * P, sl],
                            2 * c + q, f"wo{c}_{q}",
                        )

            import contextlib

            if time_loops is not None:
                emit_wv_conv()
                emit_wqk_conv()
                emit_wout_conv(range(CT))
                loop_cm = tc.For_i(0, time_loops, 1)
            else:
                loop_cm = contextlib.nullcontext()
            with loop_cm:
              d_pending = None
              for b in range(BL):
                # ------- A: hs -> hsT (bf16), two column halves -------------
                hsT = [
                    main.tile([P, T], BF16, tag=f"hsT{c}", name=f"hsT{b}_{c}")
                    for c in range(CT)
                ]
                for thalf in range(2):
                    if thalf == 1 and b == 0 and time_loops is None:
                        emit_wv_conv()
                    for chalf in range(2):
                        # previous item's out-projection chunks fill the PE
                        # while this block's DMA+convert chain is in flight
                        if d_pending is not None:
                            blk = thalf * 2 + chalf
                            for ch in d_pending[blk * 4 : blk * 4 + 4]:
                                ch()
                        hs_bf = []
                        for i in range(4):
                            t = thalf * 4 + i
                            h_t = pipe.tile([P, NQ], F32, tag=f"hsl{i}",
                                            bufs=2, name=f"hs{b}_{t}_{chalf}")
                            dma_eng = nc.sync if i % 2 == 0 else nc.gpsimd
                            dma_eng.dma_start(
                                out=h_t,
                                in_=hs[b, t * P : (t + 1) * P,
                                       chalf * NQ : (chalf + 1) * NQ],
                            )
                            h_bf = pipe.tile([P, NQ], BF16, tag=f"hsb{i}",
                                             bufs=1, name=f"hsb{b}_{t}_{chalf}")
                            nc.scalar.activation(h_bf, h_t, Copy)
                            hs_bf.append(h_bf)
                        for cc in range(4):
                            c = chalf * 4 + cc
                            ps_tr = psum.tile([P, NQ], BF16, tag="p_qk", bufs=2,
                                              name=f"ps_tr{b}_{thalf}_{c}")
                            for i in range(4):
                                nc.tensor.transpose(
                                    ps_tr[:, i * P : (i + 1) * P],
                                    hs_bf[i][:, cc * P : (cc + 1) * P],
                                    ident_bf,
                                )
                            nc.vector.tensor_copy(
                                hsT[c][:, thalf * NQ : (thalf + 1) * NQ], ps_tr
                            )

                # ------- V-projection: V~[kt] = [k, 16*(64+1)] bf16 ---------
                if upto == "hsT":
                    continue
                V = []
                for t in range(TT):
                    ps_v = psum.tile([P, D], F32, tag="p_l", bufs=2,
                                     name=f"ps_v{b}_{t}")
                    for c in range(CT):
                        for q in range(2):
                            sl = slice(q * NQ, (q + 1) * NQ)
                            nc.tensor.matmul(
                                ps_v[:, sl],
                                hsT[c][:, t * P : (t + 1) * P],
                                wv_bf[c][:, sl],
                                start=(c == 0), stop=(c == CT - 1),
                            )
                    v_t = main.tile([P, H * HDe], BF16, tag=f"v{t}",
                                    name=f"V{b}_{t}")
                    v3 = v_t.rearrange("p (h e) -> p h e", h=H)
                    nc.vector.tensor_copy(v3[:, :, HD:HDe], ones_ph)
                    nc.vector.tensor_tensor(
                        out=v3[:, :, 0:HD],
                        in0=ps_v.rearrange("p (h e) -> p h e", h=H),
                        in1=bcast_bv.rearrange("p (h e) -> p h e", h=H),
                        op=ADD,
                    )
                    V.append(v_t)

                # ------- C: fine-grained interleaved pair pipeline ----------
                # Per kt-step of pair p the PE stream gets: logits(p, kt)
                # [ACT-bound consumer], two AV accumulation chains of pair
                # p-1, and (every other step) a QK-projection half-job of
                # pair p+1 -- so the in-order PE never parks on the ps_l WAR
                # against the exp drain.
                attnT = [
                    main.tile([P, T], BF16, tag=f"at{g}", name=f"attnT{b}_{g}")
                    for g in range(CT)
                ]

                def make_qk_jobs(hp):
                    dsts = {
                        w: pipe.tile([P, T], BF16, tag=f"{w}t", bufs=2,
                                     name=f"{w.upper()}T{b}_{hp}")
                        for w in ("q", "k")
                    }
                    jobs = []
                    for which, jt in (("q", hp), ("k", CT + hp)):
                        for qh in range(2):
                            def job(which=which, jt=jt, qh=qh):
                                sl = slice(qh * NQ, (qh + 1) * NQ)
                                ps = psum.tile([P, NQ], F32, tag="p_qk",
                                               bufs=2,
                                               name=f"ps_qk{b}_{jt}_{qh}")
                                for c in range(CT):
                                    nc.tensor.matmul(
                                        ps, wqk_bf[c][:, jt, :],
                                        hsT[c][:, sl],
                                        start=(c == 0), stop=(c == CT - 1),
                                    )
                                nc.vector.tensor_scalar_add(
                                    dsts[which][:, sl], ps, bq[:, jt : jt + 1]
                                )
                            jobs.append(job)
                    return jobs, dsts

                def emit_logits_step(hp, dsts, kt, exps):
                    # q-major MM order: adjacent MMs use disjoint 64-row
                    # groups (head A rows 0:64, head B 64:128), so the PE
                    # runs each adjacent pair concurrently
                    QTg, KTg = dsts["q"], dsts["k"]
                    ps_ls = [
                        psum.tile([P, T], F32, tag="p_l", bufs=2,
                                  name=f"ps_l{b}_{hp}_{kt}_{i}")
                        for i in range(2)
                    ]
                    for q in range(2):
                        sl = slice(q * NQ, (q + 1) * NQ)
                        for i in range(2):
                            r0 = i * HD
                            nc.tensor.matmul(
                                ps_ls[i][:, sl],
                                KTg[r0 : r0 + HD, kt * P : (kt + 1) * P],
                                QTg[r0 : r0 + HD, sl],
                                start=True, stop=True,
                            )
                    for i in range(2):
                        expt = pipe.tile([P, T], BF16, tag=f"exp{i}", bufs=12,
                                         name=f"exp{b}_{hp}_{kt}_{i}")
                        nc.scalar.activation(expt, ps_ls[i], Exp,
                                             scale=float(SCALE))
                        exps[i].append(expt)

                def make_av_schedule(hp, exps, delay_trans=False):
                    """Returns per-step emitters [0..7] + tail for pair hp's
                    AV/norm/transpose. Chain groups (i, grp) in order
                    (0,0),(1,0),(0,1),(1,1); group n occupies steps 2n,2n+1
                    (two 8-MM chains per step). Norms emitted two steps after
                    a group starts; packed 4-transposes of q-group g go at
                    step 5 (g=0) and the tail (g=1)."""
                    a_pair = [
                        pipe.tile([P, P], BF16, tag="apair", bufs=8,
                                  name=f"ap{b}_{hp}_{qt}")
                        for qt in range(TT)
                    ]
                    groups = [(0, 0), (1, 0), (0, 1), (1, 1)]
                    ps_avs = {}

                    def chains(step):
                        i, grp = groups[step // 2]
                        if step % 2 == 0:
                            ps_avs[(i, grp)] = psum.tile(
                                [P, 4 * HDe], F32, tag="p_av", bufs=2,
                                name=f"ps_av{b}_{hp}_{i}_{grp}",
                            )
                        ps_av = ps_avs[(i, grp)]
                        h = 2 * hp + i
                        for ql in (0, 1) if step % 2 == 0 else (2, 3):
                            qt = grp * 4 + ql
                            for kt in range(TT):
                                nc.tensor.matmul(
                                    ps_av[:, ql * HDe : (ql + 1) * HDe],
                                    exps[i][kt][:, qt * P : (qt + 1) * P],
                                    V[kt][:, h * HDe : (h + 1) * HDe],
                                    start=(kt == 0), stop=(kt == TT - 1),
                                )

                    def norm(i, grp):
                        h = 2 * hp + i
                        ps_av = ps_avs[(i, grp)]
                        recips = pipe.tile([P, 4], F32, tag="recip", bufs=4,
                                           name=f"rc{b}_{h}_{grp}")
                        nc.vector.reciprocal(recips, ps_av[:, HD :: HDe])
                        for ql in range(4):
                            qt = grp * 4 + ql
                            nc.vector.tensor_scalar_mul(
                                a_pair[qt][:, i * HD : (i + 1) * HD],
                                ps_av[:, ql * HDe : ql * HDe + HD],
                                recips[:, ql : ql + 1],
                            )

                    def trans(grp):
                        ps_t4 = psum.tile([P, NQ], BF16, tag="p_av", bufs=2,
                                          name=f"ps_at{b}_{hp}_{grp}")
                        for ql in range(4):
                            qt = grp * 4 + ql
                            nc.tensor.transpose(
                                ps_t4[:, ql * P : (ql + 1) * P],
                                a_pair[qt], ident_bf,
                            )
                        nc.vector.tensor_copy(
                            attnT[hp][:, grp * NQ : (grp + 1) * NQ], ps_t4
                        )

                    def step(s):
                        if s == 2:
                            norm(0, 0)
                        elif s == 4:
                            norm(1, 0)
                        elif s == 5 and not delay_trans:
                            trans(0)
                        elif s == 6:
                            norm(0, 1)
                        chains(s)

                    def tail():
                        norm(1, 1)
                        if delay_trans:
                            trans(0)
                        trans(1)

                    return step, tail

                def make_d_chunks(bb, attnT_b):
                    # out-projection as 16 half-jobs of 8 MMs; interleaved
                    # into the NEXT item's pair-0/1 steps to fill the PE's
                    # idle while ACT paces the exp chain
                    o_ts = {}

                    def chunk(t, e, bb=bb, attnT_b=attnT_b):
                        sl = slice(e * NQ, (e + 1) * NQ)
                        ps_oh = psum.tile([P, NQ], F32, tag="p_av", bufs=2,
                                          name=f"ps_o{bb}_{t}_{e}")
                        for dt in range(CT):
                            nc.tensor.matmul(
                                ps_oh,
                                attnT_b[dt][:, t * P : (t + 1) * P],
                                wout_bf[dt][:, sl],
                                start=(dt == 0), stop=(dt == CT - 1),
                            )
                        if e == 0:
                            o_ts[t] = pipe.tile([P, D], F32, tag="ot",
                                                bufs=2, name=f"o{bb}_{t}")
                        nc.vector.tensor_tensor(
                            out=o_ts[t][:, sl], in0=ps_oh,
                            in1=bcast_bout[:, sl], op=ADD
                        )
                        if e == 1:
                            (nc.gpsimd if t % 2 == 0 else nc.sync).dma_start(
                                out=out[bb, t * P : (t + 1) * P, :],
                                in_=o_ts[t],
                            )

                    return [
                        (lambda t=t, e=e: chunk(t, e))
                        for t in range(TT) for e in range(2)
                    ]

                if upto == "vproj":
                    continue
                if b == 0 and time_loops is None:
                    emit_wqk_conv()
                # warm-up: pair 0's QK projection (V-phase keeps PE dense)
                jobs, dsts_cur = make_qk_jobs(0)
                for j in jobs:
                    j()
                if upto == "qk":
                    for hp in range(1, H // 2):
                        jobs, _ = make_qk_jobs(hp)
                        for j in jobs:
                            j()
                    continue

                av_step, av_tail = None, None
                dsts_nxt, jobs_nxt = None, None
                for hp in range(H // 2):
                    exps = [[], []]
                    if hp + 1 < H // 2:
                        jobs_nxt, dsts_nxt = make_qk_jobs(hp + 1)
                    else:
                        jobs_nxt = None
                    if b == 0 and time_loops is None and 1 <= hp <= 4:
                        emit_wout_conv(range(2 * (hp - 1), 2 * hp))
                    for kt in range(TT):
                        emit_logits_step(hp, dsts_cur, kt, exps)
                        if av_step is not None:
                            av_step(kt)
                        if jobs_nxt is not None and kt % 2 == 0:
                            jobs_nxt[kt // 2]()
                    if av_tail is not None:
                        av_tail()
                    if upto != "logits":
                        av_step, av_tail = make_av_schedule(hp, exps)
                    dsts_cur = dsts_nxt
                d_pending = None
                if upto == "logits":
                    continue
                # last pair's AV, un-interleaved
                for s in range(TT):
                    av_step(s)
                av_tail()
                if upto == "av":
                    continue

                # ------- D: out projection (deferred into next item) --------
                d_chunks = make_d_chunks(b, attnT)
                if b == BL - 1:
                    for ch in d_chunks:
                        ch()
                else:
                    d_pending = d_chunks

    _dedupe_ldweights(nc)
    nc.compile()
    return nc


def _dedupe_ldweights(nc):
    """Remove InstLdweights whose stationary operand is already resident in
    the same PE row group (e.g. the q-half reload in each logits step: the
    A,B,A,B row-group pattern re-loads both K=64 stationaries although the
    disjoint row groups keep them resident). Waits from a removed load are
    carried onto the next PE instruction."""
    removed = 0
    for blk in nc.m.functions[0].blocks:
        insts = list(blk.instructions)
        loaded = {}  # (row0, nrows) -> payload key
        pending_waits = []
        keep = []
        for inst in insts:
            nm = type(inst).__name__
            if nm == "InstLdweights":
                tp = inst.tile_position or (0, 0)
                ts = inst.tile_size
                nrows = ts[0] if ts else 128
                rows = (tp[0], nrows)
                key = (
                    str(inst.ins[0]), tuple(tp), str(ts),
                    str(inst.perf_mode), str(inst.is_transpose),
                )
                if loaded.get(rows) == key:
                    si = inst.sync_info
                    if si is not None and si.on_wait:
                        pending_waits.extend(si.on_wait)
                    removed += 1
                    continue
                for r in list(loaded):
                    if not (r[0] + r[1] <= rows[0] or rows[0] + rows[1] <= r[0]):
                        del loaded[r]
                loaded[rows] = key
                keep.append(inst)
            else:
                if nm == "InstMatmult" and pending_waits:
                    si = inst.sync_info
                    if si is None:
                        inst.sync_info = mybir.SyncInfo(
                            on_wait=list(pending_waits), on_update=[]
                        )
                    else:
                        si.on_wait = list(si.on_wait) + pending_waits
                    pending_waits = []
                keep.append(inst)
        if removed and len(keep) != len(insts):
            while len(blk.instructions):
                blk.instructions.pop()
            for inst in keep:
                blk.instructions.append(inst)
    return removed


_nc_cache = None


def kernel(**inputs) -> np.ndarray:
    global _nc_cache, _last_results
    hs = np.ascontiguousarray(np.asarray(inputs["hidden_states"], dtype=np.float32))
    w_qkv = np.ascontiguousarray(np.asarray(inputs["w_qkv"], dtype=np.float32))
    b_qkv = np.ascontiguousarray(np.asarray(inputs["b_qkv"], dtype=np.float32))
    w_out = np.ascontiguousarray(np.asarray(inputs["w_out"], dtype=np.float32))
    b_out = np.ascontiguousarray(np.asarray(inputs["b_out"], dtype=np.float32))

    if _nc_cache is None:
        _nc_cache = build_program()
    nc = _nc_cache

    in_maps = [
        {
            "hidden_states": hs[c * BL : (c + 1) * BL],
            "w_qkv": w_qkv,
            "b_qkv": b_qkv,
            "w_out": w_out,
            "b_out": b_out,
        }
        for c in range(N_CORES)
    ]
    try:
        res = run_bass_kernel_spmd(
            nc,
            in_maps,
            list(range(N_CORES)),
            trace=bool(os.environ.get("BASS_TRACE")),
        )
    except ModuleNotFoundError:
        prev = os.environ.get("BASS_NEVER_TRACE")
        os.environ["BASS_NEVER_TRACE"] = "1"
        try:
            res = run_bass_kernel_spmd(nc, in_maps, list(range(N_CORES)))
        finally:
            if prev is None:
                os.environ.pop("BASS_NEVER_TRACE", None)
            else:
                os.environ["BASS_NEVER_TRACE"] = prev
    _last_results = res
    return np.concatenate([res.results[c]["out"] for c in range(N_CORES)], axis=0)

